# revision 18
# baseline (speedup 1.0000x reference)
"""Trainium2 Bass kernel for a binarized (XNOR-style) ResNet BasicBlock.

Reference semantics (per nn_BasicBlock_37228776522124):
    out = BN2(conv3x3(sign(BN1(conv3x3(sign(x), sign(w1)*a1))), sign(w2)*a2)) + x
with training-mode BN (batch stats over N,H,W) and per-out-channel
weight scale a_l = mean(|w_l|).

Key facts exploited:
  * conv inputs are exactly +-1 -> fp8 DoubleRow matmuls accumulate EXACT
    integers in fp32 PSUM (|z| <= 2304 < 2^24).
  * conv(sign(x), sign(w)*a) = a * conv(sign(x), sign(w)); a and BN fold
    into one per-channel affine s*z + b applied post-conv.
  * z is always even; z/2 <= 1152 is stored exactly in fp16. conv1's z
    only feeds sign(z - mean), so it is stored as fp8 at z/16.
  * Data-parallel over batch (4 images/core on 8 cores); BN batch stats
    need one AllReduce of [128,4] fp32 per conv.

Pipeline layout (v3):
  * startup: x(n0), w1 chunks ride three parallel DMA paths (sync /
    scalar / gpsimd-SWDGE); weight-sign evacuation runs on the DVE
    (2-op is_ge*2-1) so the ACT queue goes straight to the conv fills.
  * identity residual: x is copied DRAM->DRAM into `out` during conv1;
    the finalize DMA-accumulates s2*z2+b2 on top (gpsimd accum_op=add),
    so no x bytes cross SBUF in the tail.
  * finalize is chunked in 16 half-tiles alternating ACT activation and
    DVE tensor_scalar for the affine, 6-deep buffering.

Self-contained: only needs /opt/trn_rl_repo (the Bass toolchain) + numpy.
"""

import os
import sys

for _p in ("/opt/trn_rl_repo",):
    if os.path.isdir(_p) and _p not in sys.path:
        sys.path.insert(0, _p)

import numpy as np

# Problem shapes (hardcoded per spec)
N_FULL, C, H, W = 32, 256, 56, 56
NCORES = 8
NPER = N_FULL // NCORES          # 4 images per core
SP = H * W                       # 3136
HSP = SP // 2                    # finalize half-tile
HP = H + 2                       # 58 (zero-padded)
SPP = HP * HP                    # 3364
NIB = C // 128                   # 2 input-channel blocks
NOB = C // 128                   # 2 output-channel blocks
NTAP = 9
NK = NTAP * NIB                  # 18 accumulation steps per output tile
RB = 7                           # row-blocks of 8 rows
RBW = 8 * W                      # 448 valid outputs per row-block
NMOV = 8 * HP                    # 464 moving columns (8 contiguous pad rows)
RBQ = NMOV + 2                   # 466 f32 <= one psum bank
EPS = 1e-5
KELEM = C * NTAP                 # 2304 weight elems per out channel
KH = KELEM // 2                  # ib-half of a weight row (contiguous)

_nc_cache = {}


def build_nc(num_devices=NCORES):
    import concourse.bacc as bacc
    import concourse.tile as tile
    import concourse.mybir as mybir
    from concourse.masks import make_identity

    F32 = mybir.dt.float32
    F16 = mybir.dt.float16
    BF16 = mybir.dt.bfloat16
    ALU = mybir.AluOpType
    ACTF = mybir.ActivationFunctionType
    AX = mybir.AxisListType

    nc = bacc.Bacc(
        "TRN2", target_bir_lowering=False, debug=False,
        num_devices=num_devices,
    )

    x_t = nc.dram_tensor("x", [NPER, C, H, W], F32, kind="ExternalInput")
    w_t = [
        nc.dram_tensor("w1", [C, C, 3, 3], F32, kind="ExternalInput"),
        nc.dram_tensor("w2", [C, C, 3, 3], F32, kind="ExternalInput"),
    ]
    g_t = [
        nc.dram_tensor("gamma1", [C], F32, kind="ExternalInput"),
        nc.dram_tensor("gamma2", [C], F32, kind="ExternalInput"),
    ]
    b_t = [
        nc.dram_tensor("beta1", [C], F32, kind="ExternalInput"),
        nc.dram_tensor("beta2", [C], F32, kind="ExternalInput"),
    ]
    out_t = nc.dram_tensor("out", [NPER, C, H, W], F32, kind="ExternalOutput")

    x_ap = x_t.ap().rearrange("n c h w -> n c (h w)")      # [4, 256, 3136]
    out_ap = out_t.ap().rearrange("n c h w -> n c (h w)")
    rgroups = [list(range(num_devices))]
    M_TOTAL = float(num_devices * NPER * SP)
    # debug bisection: W < C1 < AR1 < C2 < FULL
    phase_lim = {"W": 0, "C1": 1, "AR1": 2, "C2": 3, "FULL": 9}[
        os.environ.get("KERNEL_PHASES", "FULL")]
    use_fp8 = os.environ.get("KERNEL_FP8", "1") == "1"
    A8 = mybir.dt.float8e4
    PM = mybir.MatmulPerfMode
    # abuf block pitch: 2-col left margin (first-tap 466-wide matmul reads
    # from grid-2) + 3364 grid + tail pad; 3376 keeps fp8 pair-step 16B-aligned
    ABW = 3376
    GB = 2                          # grid base offset inside each block
    ABD = A8 if use_fp8 else BF16

    with tile.TileContext(nc) as tc:
        with (
            tc.tile_pool(name="consts", bufs=1) as pc,
            tc.tile_pool(name="dbl", bufs=2) as pd,
            tc.tile_pool(name="psum", bufs=8, space="PSUM") as pp,
            tc.tile_pool(name="dram", bufs=1, space="DRAM") as pdram,
        ):
            ident = pc.tile([128, 128], F32, name="ident", tag="ident")
            make_identity(nc, ident[:])
            epsap = pc.tile([128, 1], F32, name="epsap", tag="epsap")
            nc.vector.memset(epsap[:], EPS)

            # persistent stores: z1 as fp8 at z/16 (only feeds sign(z-mean)),
            # z2 as fp16 at z/2 (exact, feeds the output).
            zstore = [
                pc.tile([128, NPER * NOB * SP], A8 if l == 0 else F16,
                        name=f"z{l}", tag=f"z{l}")
                for l in range(2)
            ]
            zscale = [1.0 / 16.0, 0.5]
            wsign = [
                pc.tile([128, NK * NOB * 128], ABD, name=f"ws{l}", tag=f"ws{l}")
                for l in range(2)
            ]
            # full residual x resident as fp16 (exact to ~5e-4 rel; the
            # identity add tolerance is ~2e-3): loaded by gpsimd casting
            # DMAs paced through conv2, so the finalize only WRITES HBM.
            xf16 = pc.tile([128, NPER * NOB * SP], F16, name="xf16",
                           tag="xf16")
            alphar = [pc.tile([128, NOB], F32, name=f"al{l}", tag=f"al{l}") for l in range(2)]
            sumc = [pc.tile([128, NOB * 28], F32, name=f"sc{l}", tag=f"sc{l}") for l in range(2)]
            sqc = [pc.tile([128, NOB * 28], F32, name=f"qc{l}", tag=f"qc{l}") for l in range(2)]
            statloc = [pc.tile([128, 4], F32, name=f"sl{l}", tag=f"sl{l}") for l in range(2)]
            statg = [pc.tile([128, 4], F32, name=f"sg{l}", tag=f"sg{l}") for l in range(2)]
            gb = [pc.tile([128, 2 * NOB], F32, name=f"gb{l}", tag=f"gb{l}") for l in range(2)]
            coef = [pc.tile([128, 2 * NOB], F32, name=f"cf{l}", tag=f"cf{l}") for l in range(2)]
            btmp = [pc.tile([128, 14], F32, name=f"bt{l}", tag=f"bt{l}") for l in range(2)]

            # dummy AllReduce at kernel start: absorbs the first-collective
            # latency concurrently with conv1 so the real AR1 is fast
            ard_i = pdram.tile([128, 1], F32, name="ard_i", tag="ard_i")
            ard_o = pdram.tile([128, 1], F32, name="ard_o", tag="ard_o")
            nc.sync.dma_start(ard_i[:], g_t[0].ap()[0:128])
            nc.gpsimd.collective_compute(
                "AllReduce", ALU.add, replica_groups=rgroups,
                ins=[ard_i.opt()], outs=[ard_o.opt()],
            )
            # park the (unused) result in a spare btmp column so DCE keeps it
            nc.gpsimd.dma_start(btmp[0][:, 12:13], ard_o[:])

            # ---------- startup DMAs on three parallel queues ----------
            HROW = H // 2  # 28 rows per half

            def fill1(n, abuf):
                # x in half-tiles through a 3-deep chunk pool: finer DMA
                # pacing and the sign pass starts after the first half
                for hh in range(2):
                    for ib in range(NIB):
                        a58 = abuf[:, ib * ABW + GB:ib * ABW + GB + SPP
                                   ].rearrange("p (h w) -> p h w", w=HP)
                        xc = pd.tile([128, HSP], F32, name="xin", tag="xin",
                                     bufs=3)
                        nc.sync.dma_start(
                            xc[:], x_ap[n, ib * 128:(ib + 1) * 128,
                                        hh * HSP:(hh + 1) * HSP])
                        xv = xc[:].rearrange("p (h w) -> p h w", w=W)
                        nc.scalar.activation(
                            out=a58[:, 1 + hh * HROW:1 + (hh + 1) * HROW,
                                    1:W + 1],
                            in_=xv, func=ACTF.Sign)

            # scalar: w1 ob0 in ib-halves; gpsimd (SWDGE): w1 ob1 halves
            wd0 = w_t[0].ap().rearrange("o i h w -> o (i h w)")
            wraw0 = []
            for ob in range(NOB):
                wr = pc.tile([128, KELEM], F32, name="wraw", tag="wraw",
                             bufs=2)
                eng = nc.scalar if ob == 0 else nc.gpsimd
                for ibh in range(2):
                    eng.dma_start(
                        wr[:, ibh * KH:(ibh + 1) * KH],
                        wd0[ob * 128:(ob + 1) * 128,
                            ibh * KH:(ibh + 1) * KH])
                wraw0.append(wr)
            for ob in range(NOB):
                nc.scalar.dma_start(
                    gb[0][:, ob:ob + 1], g_t[0].ap()[ob * 128:(ob + 1) * 128])
                nc.scalar.dma_start(
                    gb[0][:, NOB + ob:NOB + ob + 1],
                    b_t[0].ap()[ob * 128:(ob + 1) * 128])

            # ---------------- weight prep helpers ----------------
            def wprep_tap(l, wraw, ob, t, ib, dve=False):
                """transpose one (ob, t, ib) 128x128 block + sign-evacuate.

                dve=True: 2-op DVE evacuation (is_ge -> {0,1}, then *2-1),
                keeps the ACT queue free. dve=False: single ACT Sign."""
                wtap = wraw[:].rearrange("p (i t) -> p t i", t=NTAP)
                if use_fp8:
                    kidx = (ob * NTAP + t) * 2 + ib
                else:
                    kidx = ob * NK + t * NIB + ib
                dst = wsign[l][:, kidx * 128:(kidx + 1) * 128]
                psT = pp.tile([128, RBW], F32, name="cps", tag="cps")
                nc.tensor.transpose(
                    psT[:, 0:128],
                    wtap[:, t, ib * 128:(ib + 1) * 128],
                    ident[:],
                )
                if dve:
                    # {0,2} = (wT >= 0)*2, then in-place -1 -> exact +-1 fp8
                    nc.vector.tensor_scalar(
                        out=dst, in0=psT[:, 0:128],
                        scalar1=0.0, scalar2=2.0, op0=ALU.is_ge, op1=ALU.mult,
                    )
                    nc.vector.tensor_scalar_add(dst, dst, -1.0)
                else:
                    nc.scalar.activation(
                        out=dst, in_=psT[:, 0:128], func=ACTF.Sign)

            def wprep_alpha(l, wraw, ob):
                nc.vector.tensor_reduce(
                    out=alphar[l][:, ob:ob + 1], in_=wraw[:],
                    axis=AX.X, op=ALU.add, apply_absolute_value=True,
                )

            # conv1 weight prep: transposes paced by DVE sign-evacuation,
            # ordered by DMA chunk arrival (ob0ib0, ob0ib1, ob1ib0, ob1ib1)
            for ob in range(NOB):
                for ib in range(NIB):
                    for t in range(NTAP):
                        wprep_tap(0, wraw0[ob], ob, t, ib, dve=True)

            # fill1 for n=0 (emitted now; ACT goes straight to it while
            # the DVE paces the weight-sign evacuation)
            abuf0 = pd.tile([128, NIB * ABW], ABD, name="abuf", tag="abuf")
            for ib in range(NIB):
                a58 = abuf0[:, ib * ABW + GB:ib * ABW + GB + SPP
                            ].rearrange("p (h w) -> p h w", w=HP)
                nc.vector.memset(a58[:, 0:1, :], 0.0)
                nc.vector.memset(a58[:, HP - 1:HP, :], 0.0)
                nc.vector.memset(a58[:, :, 0:1], 0.0)
                nc.vector.memset(a58[:, :, HP - 1:HP], 0.0)
                nc.vector.memset(abuf0[:, ib * ABW:ib * ABW + GB], 0.0)
                nc.vector.memset(
                    abuf0[:, ib * ABW + GB + SPP:(ib + 1) * ABW], 0.0)
            fill1(0, abuf0)

            for ob in range(NOB):
                wprep_alpha(0, wraw0[ob], ob)

            # ---------------- one conv pass (shared for conv1/conv2) --------
            def conv_pass(l, act_fill, do_ar=True, abuf_pre=None,
                          post_group=None):
                """act_fill(n, abuf) writes signed acts into the padded
                [128, NIB*SPP] buffer interior (ring already zero).
                post_group(n, ob) emits extra work after each group."""
                def make_abuf(n):
                    abuf = pd.tile([128, NIB * ABW], ABD, name="abuf",
                                   tag="abuf")
                    for ib in range(NIB):
                        a58 = abuf[:, ib * ABW + GB:ib * ABW + GB + SPP
                                   ].rearrange("p (h w) -> p h w", w=HP)
                        nc.vector.memset(a58[:, 0:1, :], 0.0)
                        nc.vector.memset(a58[:, HP - 1:HP, :], 0.0)
                        nc.vector.memset(a58[:, :, 0:1], 0.0)
                        nc.vector.memset(a58[:, :, HP - 1:HP], 0.0)
                        nc.vector.memset(abuf[:, ib * ABW:ib * ABW + GB], 0.0)
                        nc.vector.memset(
                            abuf[:, ib * ABW + GB + SPP:(ib + 1) * ABW], 0.0)
                    act_fill(n, abuf)
                    return abuf

                pend = {}
                for n in range(NPER):
                    if n == 0 and abuf_pre is not None:
                        abuf = abuf_pre
                    elif n in pend:
                        abuf = pend.pop(n)
                    else:
                        abuf = make_abuf(n)
                    # emit the NEXT image's fill ahead of this image's
                    # drains so the ACT queue never parks it behind the
                    # PE-gated square passes
                    if n + 1 < NPER:
                        pend[n + 1] = make_abuf(n + 1)
                    for ob in range(NOB):
                        ps = [pp.tile([128, RBQ], F32, name="cps", tag="cps")
                              for _ in range(RB)]
                        if use_fp8:
                            ab3 = abuf[:].rearrange(
                                "p (two s) -> p two s", two=NIB)
                            for t in range(NTAP):
                                th, tw = t // 3, t % 3
                                base = (ob * NTAP + t) * 2 * 128
                                lhsT = wsign[l][:, base:base + 256].rearrange(
                                    "p (two m) -> p two m", two=2)
                                for rb in range(RB):
                                    r0 = (rb * 8 + th) * HP
                                    if t == 0:
                                        rhs = ab3[:, :, r0:r0 + RBQ]
                                        outap = ps[rb][:, 0:RBQ]
                                    else:
                                        rhs = ab3[:, :, GB + r0:GB + r0 + NMOV]
                                        outap = ps[rb][:, 2 - tw:2 - tw + NMOV]
                                    nc.tensor.matmul(
                                        outap, lhsT, rhs,
                                        start=(t == 0), stop=(t == NTAP - 1),
                                        perf_mode=PM.DoubleRow,
                                    )
                        else:
                            for k in range(NK):
                                t, ib = k // NIB, k % NIB
                                th, tw = t // 3, t % 3
                                kidx = ob * NK + t * NIB + ib
                                af = abuf[:, ib * ABW:(ib + 1) * ABW]
                                lhsT = wsign[l][:, kidx * 128:(kidx + 1) * 128]
                                for rb in range(RB):
                                    r0 = (rb * 8 + th) * HP
                                    if k == 0:
                                        rhs = af[:, r0:r0 + RBQ]
                                        outap = ps[rb][:, 0:RBQ]
                                    else:
                                        rhs = af[:, GB + r0:GB + r0 + NMOV]
                                        outap = ps[rb][:, 2 - tw:2 - tw + NMOV]
                                    nc.tensor.matmul(
                                        outap, lhsT, rhs,
                                        start=(k == 0),
                                        stop=(k == NK - 1),
                                    )
                        zs = zstore[l]
                        for rb in range(RB):
                            col = n * RB + rb
                            zsl = zs[:, ((n * NOB + ob) * SP + rb * RBW):
                                      ((n * NOB + ob) * SP + (rb + 1) * RBW)
                                      ].rearrange("p (h w) -> p h w", w=W)
                            qv = ps[rb][:, 2:2 + NMOV].rearrange(
                                "p (h w) -> p h w", w=HP)[:, :, 0:W]
                            # z*zscale -> store on DVE; accum_out = sum
                            nc.vector.tensor_scalar(
                                out=zsl, in0=qv,
                                scalar1=zscale[l], scalar2=None, op0=ALU.mult,
                                op1=ALU.add,
                                accum_out=sumc[l][:, ob * 28 + col:
                                                  ob * 28 + col + 1],
                            )
                            # scr = z^2 (bf16 dummy out); accum = sum(z^2)
                            scr = pd.tile([128, RBW], BF16, name="scr",
                                          tag="scr")
                            nc.scalar.activation(
                                out=scr[:].rearrange("p (h w) -> p h w", w=W),
                                in_=qv, func=ACTF.Square,
                                accum_out=sqc[l][:, ob * 28 + col:
                                                 ob * 28 + col + 1],
                            )
                        if post_group is not None:
                            post_group(n, ob)
                # local stats -> [sum_ob0, sum_ob1, sq_ob0, sq_ob1]
                for ob in range(NOB):
                    nc.vector.tensor_reduce(
                        out=statloc[l][:, ob:ob + 1],
                        in_=sumc[l][:, ob * 28:(ob + 1) * 28],
                        axis=AX.X, op=ALU.add,
                    )
                    nc.vector.tensor_reduce(
                        out=statloc[l][:, NOB + ob:NOB + ob + 1],
                        in_=sqc[l][:, ob * 28:(ob + 1) * 28],
                        axis=AX.X, op=ALU.add,
                    )
                if not do_ar:
                    return
                # AllReduce across cores (DRAM bounce). AR DMAs ride the
                # scalar queue for l=1 so the sync queue never blocks them.
                dma_eng = nc.sync if l == 0 else nc.scalar
                arin = pdram.tile([128, 4], F32, name=f"ari{l}", tag=f"ari{l}")
                arout = pdram.tile([128, 4], F32, name=f"aro{l}", tag=f"aro{l}")
                dma_eng.dma_start(arin[:], statloc[l][:])
                nc.gpsimd.collective_compute(
                    "AllReduce", ALU.add, replica_groups=rgroups,
                    ins=[arin.opt()], outs=[arout.opt()],
                )
                dma_eng.dma_start(statg[l][:], arout[:])
                # BN fold: coef = [s' | beta - s*mean] per ob column
                tmp = btmp[l]
                for ob in range(NOB):
                    mean = tmp[:, 0 + ob * 6:1 + ob * 6]
                    e2 = tmp[:, 1 + ob * 6:2 + ob * 6]
                    var = tmp[:, 2 + ob * 6:3 + ob * 6]
                    alp = tmp[:, 3 + ob * 6:4 + ob * 6]
                    tt = tmp[:, 4 + ob * 6:5 + ob * 6]
                    std = tmp[:, 5 + ob * 6:6 + ob * 6]
                    nc.vector.tensor_scalar_mul(
                        mean, statg[l][:, ob:ob + 1],
                        1.0 / (zscale[l] * M_TOTAL))
                    nc.vector.tensor_scalar_mul(
                        e2, statg[l][:, NOB + ob:NOB + ob + 1], 1.0 / M_TOTAL)
                    nc.vector.tensor_mul(var, mean, mean)
                    nc.vector.tensor_sub(var, e2, var)
                    nc.vector.tensor_scalar_mul(
                        alp, alphar[l][:, ob:ob + 1], 1.0 / KELEM)
                    nc.vector.tensor_mul(tt, alp, alp)
                    nc.vector.tensor_mul(tt, tt, var)
                    nc.scalar.activation(std, tt, ACTF.Sqrt, bias=epsap[:])
                    nc.vector.reciprocal(tt, std)
                    nc.vector.tensor_mul(tt, tt, alp)             # alpha*inv
                    nc.vector.tensor_mul(tt, tt, gb[l][:, ob:ob + 1])  # *gamma
                    nc.vector.tensor_scalar_mul(
                        coef[l][:, ob:ob + 1], tt, 1.0 / zscale[l])
                    nc.vector.tensor_mul(tt, tt, mean)
                    nc.vector.tensor_sub(
                        coef[l][:, NOB + ob:NOB + ob + 1],
                        gb[l][:, NOB + ob:NOB + ob + 1], tt)

            # ---------------- conv1: acts = sign(x) ----------------
            if phase_lim >= 1:
                conv_pass(0, fill1, do_ar=(phase_lim >= 2), abuf_pre=abuf0)

            # conv2 weight prep: PE transposes + ACT signs queue up behind
            # conv1's work and execute inside the AR1 bubble.
            wd1 = w_t[1].ap().rearrange("o i h w -> o (i h w)")
            wraw1 = []
            for ob in range(NOB):
                wr = pc.tile([128, KELEM], F32, name="wraw", tag="wraw",
                             bufs=2)
                nc.sync.dma_start(wr[:], wd1[ob * 128:(ob + 1) * 128, :])
                wraw1.append(wr)
            for ob in range(NOB):
                nc.sync.dma_start(
                    gb[1][:, ob:ob + 1], g_t[1].ap()[ob * 128:(ob + 1) * 128])
                nc.sync.dma_start(
                    gb[1][:, NOB + ob:NOB + ob + 1],
                    b_t[1].ap()[ob * 128:(ob + 1) * 128])
            for ob in range(NOB):
                for ib in range(NIB):
                    for t in range(NTAP):
                        wprep_tap(1, wraw1[ob], ob, t, ib, dve=False)
                wprep_alpha(1, wraw1[ob], ob)

            # ---------------- conv2: acts = sign(s1*z1 + b1) ----------------
            def fill2(n, abuf):
                # n=0 emitted in quarter-rows: the first matmul group only
                # needs the top rows of both ib blocks
                halves = 4 if n == 0 else 1
                HR = H // halves
                for hh in range(halves):
                    for ib in range(NIB):
                        a58 = abuf[:, ib * ABW + GB:ib * ABW + GB + SPP
                                   ].rearrange("p (h w) -> p h w", w=HP)
                        zv = zstore[0][:, (n * NOB + ib) * SP:
                                       (n * NOB + ib + 1) * SP].rearrange(
                            "p (h w) -> p h w", w=W)
                        nc.scalar.activation(
                            out=a58[:, 1 + hh * HR:1 + (hh + 1) * HR,
                                    1:W + 1],
                            in_=zv[:, hh * HR:(hh + 1) * HR, :],
                            func=ACTF.Sign,
                            scale=coef[0][:, ib:ib + 1],
                            bias=coef[0][:, NOB + ib:NOB + ib + 1],
                        )

            # residual prefetch: cast-DMA x (f32 DRAM -> fp16 SBUF) on
            # the gpsimd SWDGE queue, paced one tile per conv2 group so it
            # never congests the AR windows or the HWDGE rings.
            def xf16_load(n, ob):
                if phase_lim >= 9:
                    seg = (n * NOB + ob) * SP
                    nc.gpsimd.dma_start(
                        xf16[:, seg:seg + SP],
                        x_ap[n, ob * 128:(ob + 1) * 128, :])

            if phase_lim >= 3:
                conv_pass(1, fill2, do_ar=(phase_lim >= 9),
                          post_group=xf16_load)

            if phase_lim < 9:
                # debug: dump something touching live tiles into out
                dbg = pd.tile([128, SP], F32, name="dbg", tag="dbg")
                if phase_lim >= 1:
                    nc.vector.tensor_copy(dbg[:], zstore[0][:, 0:SP])
                else:
                    nc.vector.tensor_copy(dbg[:], wsign[0][:, 0:SP])
                nc.sync.dma_start(out_ap[0, 0:128, :], dbg[:])

            # ------------- finalize: out = s2*z2 + b2 + x --------------
            # 16 half-tiles: ACT affine -> DVE add (x from resident fp16)
            # -> HWDGE write. Tail HBM traffic is the 12.8MB of writes only.
            NCH = 2 * NPER * NOB
            for k in range(NCH if phase_lim >= 9 else 0):
                n, ob, hh = k // 4, (k // 2) % 2, k % 2
                zoff = (n * NOB + ob) * SP + hh * HSP
                t1 = pd.tile([128, HSP], F32, name="t1ch", tag="t1ch",
                             bufs=3)
                nc.scalar.activation(
                    out=t1[:],
                    in_=zstore[1][:, zoff:zoff + HSP],
                    func=ACTF.Identity,
                    scale=coef[1][:, ob:ob + 1],
                    bias=coef[1][:, NOB + ob:NOB + ob + 1],
                )
                nc.vector.tensor_add(t1[:], t1[:], xf16[:, zoff:zoff + HSP])
                nc.sync.dma_start(
                    out_ap[n, ob * 128:(ob + 1) * 128,
                           hh * HSP:(hh + 1) * HSP], t1[:])

    nc.compile()
    return nc


def _get_nc(num_devices=NCORES):
    if num_devices not in _nc_cache:
        _nc_cache[num_devices] = build_nc(num_devices)
    return _nc_cache[num_devices]


def kernel(**inputs):
    from concourse.bass_utils import run_bass_kernel_spmd

    nc = _get_nc(NCORES)
    x = np.ascontiguousarray(np.asarray(inputs["x"], dtype=np.float32))
    shared = {
        k: np.ascontiguousarray(np.asarray(inputs[k], dtype=np.float32))
        for k in ("w1", "gamma1", "beta1", "w2", "gamma2", "beta2")
    }
    in_maps = [
        {"x": x[c * NPER:(c + 1) * NPER], **shared} for c in range(NCORES)
    ]
    res = run_bass_kernel_spmd(nc, in_maps, core_ids=list(range(NCORES)))
    out = np.concatenate([r["out"] for r in res.results], axis=0)
    return out.astype(np.float32)


# revision 23
# speedup vs baseline: 1.1216x; 1.1216x over previous
"""Trainium2 Bass kernel for a binarized (XNOR-style) ResNet BasicBlock.

Reference semantics (per nn_BasicBlock_37228776522124):
    out = BN2(conv3x3(sign(BN1(conv3x3(sign(x), sign(w1)*a1))), sign(w2)*a2)) + x
with training-mode BN (batch stats over N,H,W) and per-out-channel
weight scale a_l = mean(|w_l|).

Key facts exploited:
  * conv inputs are exactly +-1 -> fp8 DoubleRow matmuls accumulate EXACT
    integers in fp32 PSUM (|z| <= 2304 < 2^24).
  * conv(sign(x), sign(w)*a) = a * conv(sign(x), sign(w)); a and BN fold
    into one per-channel affine s*z + b applied post-conv.
  * z is always even; z/2 <= 1152 is stored exactly in fp16. conv1's z
    only feeds sign(z - mean), so it is stored as fp8 at z/16.
  * Data-parallel over batch (4 images/core on 8 cores); BN batch stats
    need one AllReduce of [128,4] fp32 per conv.

Pipeline layout (v3):
  * startup: x(n0), w1 chunks ride three parallel DMA paths (sync /
    scalar / gpsimd-SWDGE); weight-sign evacuation runs on the DVE
    (2-op is_ge*2-1) so the ACT queue goes straight to the conv fills.
  * identity residual: x is copied DRAM->DRAM into `out` during conv1;
    the finalize DMA-accumulates s2*z2+b2 on top (gpsimd accum_op=add),
    so no x bytes cross SBUF in the tail.
  * finalize is chunked in 16 half-tiles alternating ACT activation and
    DVE tensor_scalar for the affine, 6-deep buffering.

Self-contained: only needs /opt/trn_rl_repo (the Bass toolchain) + numpy.
"""

import os
import sys

for _p in ("/opt/trn_rl_repo",):
    if os.path.isdir(_p) and _p not in sys.path:
        sys.path.insert(0, _p)

import numpy as np

# Problem shapes (hardcoded per spec)
N_FULL, C, H, W = 32, 256, 56, 56
NCORES = 8
NPER = N_FULL // NCORES          # 4 images per core
SP = H * W                       # 3136
HSP = SP // 2                    # finalize half-tile
HP = H + 2                       # 58 (zero-padded)
SPP = HP * HP                    # 3364
NIB = C // 128                   # 2 input-channel blocks
NOB = C // 128                   # 2 output-channel blocks
NTAP = 9
NK = NTAP * NIB                  # 18 accumulation steps per output tile
RB = 7                           # row-blocks of 8 rows
RBW = 8 * W                      # 448 valid outputs per row-block
NMOV = 8 * HP                    # 464 moving columns (8 contiguous pad rows)
RBQ = NMOV + 2                   # 466 f32 <= one psum bank
EPS = 1e-5
KELEM = C * NTAP                 # 2304 weight elems per out channel
KH = KELEM // 2                  # ib-half of a weight row (contiguous)

_nc_cache = {}


def build_nc(num_devices=NCORES):
    import concourse.bacc as bacc
    import concourse.tile as tile
    import concourse.mybir as mybir
    from concourse.masks import make_identity

    F32 = mybir.dt.float32
    F16 = mybir.dt.float16
    BF16 = mybir.dt.bfloat16
    ALU = mybir.AluOpType
    ACTF = mybir.ActivationFunctionType
    AX = mybir.AxisListType

    nc = bacc.Bacc(
        "TRN2", target_bir_lowering=False, debug=False,
        num_devices=num_devices,
    )

    x_t = nc.dram_tensor("x", [NPER, C, H, W], F32, kind="ExternalInput")
    w_t = [
        nc.dram_tensor("w1", [C, C, 3, 3], F32, kind="ExternalInput"),
        nc.dram_tensor("w2", [C, C, 3, 3], F32, kind="ExternalInput"),
    ]
    g_t = [
        nc.dram_tensor("gamma1", [C], F32, kind="ExternalInput"),
        nc.dram_tensor("gamma2", [C], F32, kind="ExternalInput"),
    ]
    b_t = [
        nc.dram_tensor("beta1", [C], F32, kind="ExternalInput"),
        nc.dram_tensor("beta2", [C], F32, kind="ExternalInput"),
    ]
    out_t = nc.dram_tensor("out", [NPER, C, H, W], F32, kind="ExternalOutput")

    x_ap = x_t.ap().rearrange("n c h w -> n c (h w)")      # [4, 256, 3136]
    out_ap = out_t.ap().rearrange("n c h w -> n c (h w)")
    rgroups = [list(range(num_devices))]
    M_TOTAL = float(num_devices * NPER * SP)
    # debug bisection: W < C1 < AR1 < C2 < FULL
    phase_lim = {"W": 0, "C1": 1, "AR1": 2, "C2": 3, "FULL": 9}[
        os.environ.get("KERNEL_PHASES", "FULL")]
    use_fp8 = os.environ.get("KERNEL_FP8", "1") == "1"
    A8 = mybir.dt.float8e4
    PM = mybir.MatmulPerfMode
    # abuf block pitch: 2-col left margin (first-tap 466-wide matmul reads
    # from grid-2) + 3364 grid + tail pad; 3376 keeps fp8 pair-step 16B-aligned
    ABW = 3376
    GB = 2                          # grid base offset inside each block
    ABD = A8 if use_fp8 else BF16

    with tile.TileContext(nc) as tc:
        with (
            tc.tile_pool(name="consts", bufs=1) as pc,
            tc.tile_pool(name="dbl", bufs=2) as pd,
            tc.tile_pool(name="psum", bufs=8, space="PSUM") as pp,
            tc.tile_pool(name="dram", bufs=1, space="DRAM") as pdram,
        ):
            ident = pc.tile([128, 128], F32, name="ident", tag="ident")
            make_identity(nc, ident[:])
            epsap = pc.tile([128, 1], F32, name="epsap", tag="epsap")
            nc.vector.memset(epsap[:], EPS)

            # persistent stores: z1 as fp8 at z/16 (only feeds sign(z-mean)),
            # z2 as fp16 at z/2 (exact, feeds the output).
            zstore = [
                pc.tile([128, NPER * NOB * SP], A8 if l == 0 else F16,
                        name=f"z{l}", tag=f"z{l}")
                for l in range(2)
            ]
            zscale = [1.0 / 16.0, 0.5]
            wsign = [
                pc.tile([128, NK * NOB * 128], ABD, name=f"ws{l}", tag=f"ws{l}")
                for l in range(2)
            ]
            # full residual x resident as fp16 (exact to ~5e-4 rel; the
            # identity add tolerance is ~2e-3): loaded by gpsimd casting
            # DMAs paced through conv2, so the finalize only WRITES HBM.
            xf16 = pc.tile([128, NPER * NOB * SP], F16, name="xf16",
                           tag="xf16")
            alphar = [pc.tile([128, NOB], F32, name=f"al{l}", tag=f"al{l}") for l in range(2)]
            sumc = [pc.tile([128, NOB * 28], F32, name=f"sc{l}", tag=f"sc{l}") for l in range(2)]
            sqc = [pc.tile([128, NOB * 28], F32, name=f"qc{l}", tag=f"qc{l}") for l in range(2)]
            statloc = [pc.tile([128, 4], F32, name=f"sl{l}", tag=f"sl{l}") for l in range(2)]
            statg = [pc.tile([128, 4], F32, name=f"sg{l}", tag=f"sg{l}") for l in range(2)]
            arin = [[None] * NOB for _ in range(2)]
            arout = [[None] * NOB for _ in range(2)]
            for l in range(2):
                for ob in range(NOB):
                    arin[l][ob] = pdram.tile(
                        [128, 2], F32, name=f"ari{l}{ob}", tag=f"ari{l}{ob}")
                    arout[l][ob] = pdram.tile(
                        [128, 2], F32, name=f"aro{l}{ob}", tag=f"aro{l}{ob}")
            gb = [pc.tile([128, 2 * NOB], F32, name=f"gb{l}", tag=f"gb{l}") for l in range(2)]
            coef = [pc.tile([128, 2 * NOB], F32, name=f"cf{l}", tag=f"cf{l}") for l in range(2)]
            btmp = [pc.tile([128, 14], F32, name=f"bt{l}", tag=f"bt{l}") for l in range(2)]

            # dummy AllReduce at kernel start: absorbs the first-collective
            # latency concurrently with conv1 so the real AR1 is fast
            ard_i = pdram.tile([128, 1], F32, name="ard_i", tag="ard_i")
            ard_o = pdram.tile([128, 1], F32, name="ard_o", tag="ard_o")
            nc.sync.dma_start(ard_i[:], g_t[0].ap()[0:128])
            nc.gpsimd.collective_compute(
                "AllReduce", ALU.add, replica_groups=rgroups,
                ins=[ard_i.opt()], outs=[ard_o.opt()],
            )
            # park the (unused) result in a spare btmp column so DCE keeps it
            nc.gpsimd.dma_start(btmp[0][:, 12:13], ard_o[:])

            # ---------- startup DMAs on three parallel queues ----------
            HROW = H // 2  # 28 rows per half

            def fill1(n, abuf):
                # x in half-tiles through a 3-deep chunk pool: finer DMA
                # pacing and the sign pass starts after the first half
                for hh in range(2):
                    for ib in range(NIB):
                        a58 = abuf[:, ib * ABW + GB:ib * ABW + GB + SPP
                                   ].rearrange("p (h w) -> p h w", w=HP)
                        xc = pd.tile([128, HSP], F32, name="xin", tag="xin",
                                     bufs=3)
                        nc.sync.dma_start(
                            xc[:], x_ap[n, ib * 128:(ib + 1) * 128,
                                        hh * HSP:(hh + 1) * HSP])
                        xv = xc[:].rearrange("p (h w) -> p h w", w=W)
                        nc.scalar.activation(
                            out=a58[:, 1 + hh * HROW:1 + (hh + 1) * HROW,
                                    1:W + 1],
                            in_=xv, func=ACTF.Sign)

            # scalar: w1 ob0 in ib-halves; gpsimd (SWDGE): w1 ob1 halves
            wd0 = w_t[0].ap().rearrange("o i h w -> o (i h w)")
            wraw0 = []
            for ob in range(NOB):
                wr = pc.tile([128, KELEM], F32, name="wraw", tag="wraw",
                             bufs=2)
                eng = nc.scalar if ob == 0 else nc.gpsimd
                for ibh in range(2):
                    eng.dma_start(
                        wr[:, ibh * KH:(ibh + 1) * KH],
                        wd0[ob * 128:(ob + 1) * 128,
                            ibh * KH:(ibh + 1) * KH])
                wraw0.append(wr)
            for ob in range(NOB):
                nc.scalar.dma_start(
                    gb[0][:, ob:ob + 1], g_t[0].ap()[ob * 128:(ob + 1) * 128])
                nc.scalar.dma_start(
                    gb[0][:, NOB + ob:NOB + ob + 1],
                    b_t[0].ap()[ob * 128:(ob + 1) * 128])

            # ---------------- weight prep helpers ----------------
            def wprep_tap(l, wraw, ob, t, ib, dve=False):
                """transpose one (ob, t, ib) 128x128 block + sign-evacuate.

                dve=True: 2-op DVE evacuation (is_ge -> {0,1}, then *2-1),
                keeps the ACT queue free. dve=False: single ACT Sign."""
                wtap = wraw[:].rearrange("p (i t) -> p t i", t=NTAP)
                if use_fp8:
                    kidx = (ob * NTAP + t) * 2 + ib
                else:
                    kidx = ob * NK + t * NIB + ib
                dst = wsign[l][:, kidx * 128:(kidx + 1) * 128]
                psT = pp.tile([128, RBW], F32, name="cps", tag="cps")
                nc.tensor.transpose(
                    psT[:, 0:128],
                    wtap[:, t, ib * 128:(ib + 1) * 128],
                    ident[:],
                )
                if dve:
                    # {0,2} = (wT >= 0)*2, then in-place -1 -> exact +-1 fp8
                    nc.vector.tensor_scalar(
                        out=dst, in0=psT[:, 0:128],
                        scalar1=0.0, scalar2=2.0, op0=ALU.is_ge, op1=ALU.mult,
                    )
                    nc.vector.tensor_scalar_add(dst, dst, -1.0)
                else:
                    nc.scalar.activation(
                        out=dst, in_=psT[:, 0:128], func=ACTF.Sign)

            def wprep_alpha(l, wraw, ob):
                nc.vector.tensor_reduce(
                    out=alphar[l][:, ob:ob + 1], in_=wraw[:],
                    axis=AX.X, op=ALU.add, apply_absolute_value=True,
                )

            # conv1 weight prep: transposes paced by DVE sign-evacuation,
            # ordered by DMA chunk arrival (ob0ib0, ob0ib1, ob1ib0, ob1ib1)
            for ob in range(NOB):
                for ib in range(NIB):
                    for t in range(NTAP):
                        wprep_tap(0, wraw0[ob], ob, t, ib, dve=True)

            # fill1 for n=0 (emitted now; ACT goes straight to it while
            # the DVE paces the weight-sign evacuation)
            abuf0 = pd.tile([128, NIB * ABW], ABD, name="abuf", tag="abuf")
            for ib in range(NIB):
                a58 = abuf0[:, ib * ABW + GB:ib * ABW + GB + SPP
                            ].rearrange("p (h w) -> p h w", w=HP)
                nc.vector.memset(a58[:, 0:1, :], 0.0)
                nc.vector.memset(a58[:, HP - 1:HP, :], 0.0)
                nc.vector.memset(a58[:, :, 0:1], 0.0)
                nc.vector.memset(a58[:, :, HP - 1:HP], 0.0)
                nc.vector.memset(abuf0[:, ib * ABW:ib * ABW + GB], 0.0)
                nc.vector.memset(
                    abuf0[:, ib * ABW + GB + SPP:(ib + 1) * ABW], 0.0)
            fill1(0, abuf0)

            for ob in range(NOB):
                wprep_alpha(0, wraw0[ob], ob)

            # ---------------- one conv pass (shared for conv1/conv2) --------
            def conv_pass(l, act_fill, do_ar=True, abuf_pre=None,
                          post_group=None):
                """act_fill(n, abuf) writes signed acts into the padded
                [128, NIB*SPP] buffer interior (ring already zero).
                post_group(n, ob) emits extra work after each group."""
                def make_abuf(n):
                    abuf = pd.tile([128, NIB * ABW], ABD, name="abuf",
                                   tag="abuf")
                    for ib in range(NIB):
                        a58 = abuf[:, ib * ABW + GB:ib * ABW + GB + SPP
                                   ].rearrange("p (h w) -> p h w", w=HP)
                        nc.vector.memset(a58[:, 0:1, :], 0.0)
                        nc.vector.memset(a58[:, HP - 1:HP, :], 0.0)
                        nc.vector.memset(a58[:, :, 0:1], 0.0)
                        nc.vector.memset(a58[:, :, HP - 1:HP], 0.0)
                        nc.vector.memset(abuf[:, ib * ABW:ib * ABW + GB], 0.0)
                        nc.vector.memset(
                            abuf[:, ib * ABW + GB + SPP:(ib + 1) * ABW], 0.0)
                    act_fill(n, abuf)
                    return abuf

                pend = {}
                for n in range(NPER):
                    if n == 0 and abuf_pre is not None:
                        abuf = abuf_pre
                    elif n in pend:
                        abuf = pend.pop(n)
                    else:
                        abuf = make_abuf(n)
                    # emit the NEXT image's fill ahead of this image's
                    # drains so the ACT queue never parks it behind the
                    # PE-gated square passes
                    if n + 1 < NPER:
                        pend[n + 1] = make_abuf(n + 1)
                    for ob in range(NOB):
                        ps = [pp.tile([128, RBQ], F32, name="cps", tag="cps")
                              for _ in range(RB)]
                        if use_fp8:
                            ab3 = abuf[:].rearrange(
                                "p (two s) -> p two s", two=NIB)
                            for t in range(NTAP):
                                th, tw = t // 3, t % 3
                                base = (ob * NTAP + t) * 2 * 128
                                lhsT = wsign[l][:, base:base + 256].rearrange(
                                    "p (two m) -> p two m", two=2)
                                for rb in range(RB):
                                    r0 = (rb * 8 + th) * HP
                                    if t == 0:
                                        rhs = ab3[:, :, r0:r0 + RBQ]
                                        outap = ps[rb][:, 0:RBQ]
                                    else:
                                        rhs = ab3[:, :, GB + r0:GB + r0 + NMOV]
                                        outap = ps[rb][:, 2 - tw:2 - tw + NMOV]
                                    nc.tensor.matmul(
                                        outap, lhsT, rhs,
                                        start=(t == 0), stop=(t == NTAP - 1),
                                        perf_mode=PM.DoubleRow,
                                    )
                        else:
                            for k in range(NK):
                                t, ib = k // NIB, k % NIB
                                th, tw = t // 3, t % 3
                                kidx = ob * NK + t * NIB + ib
                                af = abuf[:, ib * ABW:(ib + 1) * ABW]
                                lhsT = wsign[l][:, kidx * 128:(kidx + 1) * 128]
                                for rb in range(RB):
                                    r0 = (rb * 8 + th) * HP
                                    if k == 0:
                                        rhs = af[:, r0:r0 + RBQ]
                                        outap = ps[rb][:, 0:RBQ]
                                    else:
                                        rhs = af[:, GB + r0:GB + r0 + NMOV]
                                        outap = ps[rb][:, 2 - tw:2 - tw + NMOV]
                                    nc.tensor.matmul(
                                        outap, lhsT, rhs,
                                        start=(k == 0),
                                        stop=(k == NK - 1),
                                    )
                        zs = zstore[l]
                        for rb in range(RB):
                            col = n * RB + rb
                            zsl = zs[:, ((n * NOB + ob) * SP + rb * RBW):
                                      ((n * NOB + ob) * SP + (rb + 1) * RBW)
                                      ].rearrange("p (h w) -> p h w", w=W)
                            qv = ps[rb][:, 2:2 + NMOV].rearrange(
                                "p (h w) -> p h w", w=HP)[:, :, 0:W]
                            # z*zscale -> store on DVE; accum_out = sum
                            nc.vector.tensor_scalar(
                                out=zsl, in0=qv,
                                scalar1=zscale[l], scalar2=None, op0=ALU.mult,
                                op1=ALU.add,
                                accum_out=sumc[l][:, ob * 28 + col:
                                                  ob * 28 + col + 1],
                            )
                            # scr = z^2 (bf16 dummy out); accum = sum(z^2)
                            scr = pd.tile([128, RBW], BF16, name="scr",
                                          tag="scr")
                            nc.scalar.activation(
                                out=scr[:].rearrange("p (h w) -> p h w", w=W),
                                in_=qv, func=ACTF.Square,
                                accum_out=sqc[l][:, ob * 28 + col:
                                                 ob * 28 + col + 1],
                            )
                        if post_group is not None:
                            post_group(n, ob)
                        if do_ar and n == NPER - 1:
                            stats_ar(l, ob)
                if do_ar:
                    return
                # debug path (no AR): just the local reduces
                for ob in range(NOB):
                    nc.vector.tensor_reduce(
                        out=statloc[l][:, 2 * ob:2 * ob + 1],
                        in_=sumc[l][:, ob * 28:(ob + 1) * 28],
                        axis=AX.X, op=ALU.add,
                    )
                    nc.vector.tensor_reduce(
                        out=statloc[l][:, 2 * ob + 1:2 * ob + 2],
                        in_=sqc[l][:, ob * 28:(ob + 1) * 28],
                        axis=AX.X, op=ALU.add,
                    )

            def stats_ar(l, ob):
                """per-ob stats reduce + AllReduce + BN fold: ob0's
                collective runs while the PE still computes ob1's last
                group, and downstream per-ob consumers unblock early."""
                nc.vector.tensor_reduce(
                    out=statloc[l][:, 2 * ob:2 * ob + 1],
                    in_=sumc[l][:, ob * 28:(ob + 1) * 28],
                    axis=AX.X, op=ALU.add,
                )
                nc.vector.tensor_reduce(
                    out=statloc[l][:, 2 * ob + 1:2 * ob + 2],
                    in_=sqc[l][:, ob * 28:(ob + 1) * 28],
                    axis=AX.X, op=ALU.add,
                )
                # AR DMAs: sync for l=0, scalar for l=1
                dma_eng = nc.sync if l == 0 else nc.scalar
                dma_eng.dma_start(arin[l][ob][:],
                                  statloc[l][:, 2 * ob:2 * ob + 2])
                nc.gpsimd.collective_compute(
                    "AllReduce", ALU.add, replica_groups=rgroups,
                    ins=[arin[l][ob].opt()], outs=[arout[l][ob].opt()],
                )
                dma_eng.dma_start(statg[l][:, 2 * ob:2 * ob + 2],
                                  arout[l][ob][:])
                # BN fold: coef = [s' | beta - s*mean] for this ob
                tmp = btmp[l]
                mean = tmp[:, 0 + ob * 6:1 + ob * 6]
                e2 = tmp[:, 1 + ob * 6:2 + ob * 6]
                var = tmp[:, 2 + ob * 6:3 + ob * 6]
                alp = tmp[:, 3 + ob * 6:4 + ob * 6]
                tt = tmp[:, 4 + ob * 6:5 + ob * 6]
                std = tmp[:, 5 + ob * 6:6 + ob * 6]
                nc.vector.tensor_scalar_mul(
                    mean, statg[l][:, 2 * ob:2 * ob + 1],
                    1.0 / (zscale[l] * M_TOTAL))
                nc.vector.tensor_scalar_mul(
                    e2, statg[l][:, 2 * ob + 1:2 * ob + 2], 1.0 / M_TOTAL)
                nc.vector.tensor_mul(var, mean, mean)
                nc.vector.tensor_sub(var, e2, var)
                nc.vector.tensor_scalar_mul(
                    alp, alphar[l][:, ob:ob + 1], 1.0 / KELEM)
                nc.vector.tensor_mul(tt, alp, alp)
                nc.vector.tensor_mul(tt, tt, var)
                nc.scalar.activation(std, tt, ACTF.Sqrt, bias=epsap[:])
                nc.vector.reciprocal(tt, std)
                nc.vector.tensor_mul(tt, tt, alp)             # alpha*inv
                nc.vector.tensor_mul(tt, tt, gb[l][:, ob:ob + 1])  # *gamma
                nc.vector.tensor_scalar_mul(
                    coef[l][:, ob:ob + 1], tt, 1.0 / zscale[l])
                nc.vector.tensor_mul(tt, tt, mean)
                nc.vector.tensor_sub(
                    coef[l][:, NOB + ob:NOB + ob + 1],
                    gb[l][:, NOB + ob:NOB + ob + 1], tt)

            # ---------------- conv1: acts = sign(x) ----------------
            if phase_lim >= 1:
                conv_pass(0, fill1, do_ar=(phase_lim >= 2), abuf_pre=abuf0)

            # conv2 weight prep: PE transposes + ACT signs queue up behind
            # conv1's work and execute inside the AR1 bubble.
            wd1 = w_t[1].ap().rearrange("o i h w -> o (i h w)")
            wraw1 = []
            for ob in range(NOB):
                wr = pc.tile([128, KELEM], F32, name="wraw", tag="wraw",
                             bufs=2)
                nc.sync.dma_start(wr[:], wd1[ob * 128:(ob + 1) * 128, :])
                wraw1.append(wr)
            for ob in range(NOB):
                nc.sync.dma_start(
                    gb[1][:, ob:ob + 1], g_t[1].ap()[ob * 128:(ob + 1) * 128])
                nc.sync.dma_start(
                    gb[1][:, NOB + ob:NOB + ob + 1],
                    b_t[1].ap()[ob * 128:(ob + 1) * 128])
            for ob in range(NOB):
                for ib in range(NIB):
                    for t in range(NTAP):
                        wprep_tap(1, wraw1[ob], ob, t, ib, dve=False)
                wprep_alpha(1, wraw1[ob], ob)

            # ---------------- conv2: acts = sign(s1*z1 + b1) ----------------
            def fill2(n, abuf):
                # n=0 emitted in row-halves: the first matmul group only
                # needs the top rows of both ib blocks
                halves = 2 if n == 0 else 1
                HR = H // halves
                for hh in range(halves):
                    for ib in range(NIB):
                        a58 = abuf[:, ib * ABW + GB:ib * ABW + GB + SPP
                                   ].rearrange("p (h w) -> p h w", w=HP)
                        zv = zstore[0][:, (n * NOB + ib) * SP:
                                       (n * NOB + ib + 1) * SP].rearrange(
                            "p (h w) -> p h w", w=W)
                        nc.scalar.activation(
                            out=a58[:, 1 + hh * HR:1 + (hh + 1) * HR,
                                    1:W + 1],
                            in_=zv[:, hh * HR:(hh + 1) * HR, :],
                            func=ACTF.Sign,
                            scale=coef[0][:, ib:ib + 1],
                            bias=coef[0][:, NOB + ib:NOB + ib + 1],
                        )

            # residual prefetch: cast-DMA x (f32 DRAM -> fp16 SBUF) on
            # the gpsimd SWDGE queue, paced one tile per conv2 group so it
            # never congests the AR windows or the HWDGE rings.
            def xf16_load(n, ob):
                if phase_lim >= 9:
                    seg = (n * NOB + ob) * SP
                    nc.gpsimd.dma_start(
                        xf16[:, seg:seg + SP],
                        x_ap[n, ob * 128:(ob + 1) * 128, :])

            if phase_lim >= 3:
                conv_pass(1, fill2, do_ar=(phase_lim >= 9),
                          post_group=xf16_load)

            if phase_lim < 9:
                # debug: dump something touching live tiles into out
                dbg = pd.tile([128, SP], F32, name="dbg", tag="dbg")
                if phase_lim >= 1:
                    nc.vector.tensor_copy(dbg[:], zstore[0][:, 0:SP])
                else:
                    nc.vector.tensor_copy(dbg[:], wsign[0][:, 0:SP])
                nc.sync.dma_start(out_ap[0, 0:128, :], dbg[:])

            # ------------- finalize: out = s2*z2 + b2 + x --------------
            # 16 half-tiles: ACT affine -> DVE add (x from resident fp16)
            # -> HWDGE write. Tail HBM traffic is the 12.8MB of writes only.
            NCH = 2 * NPER * NOB
            for k in range(NCH if phase_lim >= 9 else 0):
                # ob-major: the ob0 chunks only need AR2a, so their writes
                # stream while ob1's collective is still completing
                ob, n, hh = k // 8, (k % 8) // 2, k % 2
                zoff = (n * NOB + ob) * SP + hh * HSP
                t1 = pd.tile([128, HSP], F32, name="t1ch", tag="t1ch",
                             bufs=3)
                nc.scalar.activation(
                    out=t1[:],
                    in_=zstore[1][:, zoff:zoff + HSP],
                    func=ACTF.Identity,
                    scale=coef[1][:, ob:ob + 1],
                    bias=coef[1][:, NOB + ob:NOB + ob + 1],
                )
                nc.vector.tensor_add(t1[:], t1[:], xf16[:, zoff:zoff + HSP])
                nc.sync.dma_start(
                    out_ap[n, ob * 128:(ob + 1) * 128,
                           hh * HSP:(hh + 1) * HSP], t1[:])

    nc.compile()
    return nc


def _get_nc(num_devices=NCORES):
    if num_devices not in _nc_cache:
        _nc_cache[num_devices] = build_nc(num_devices)
    return _nc_cache[num_devices]


def kernel(**inputs):
    from concourse.bass_utils import run_bass_kernel_spmd

    nc = _get_nc(NCORES)
    x = np.ascontiguousarray(np.asarray(inputs["x"], dtype=np.float32))
    shared = {
        k: np.ascontiguousarray(np.asarray(inputs[k], dtype=np.float32))
        for k in ("w1", "gamma1", "beta1", "w2", "gamma2", "beta2")
    }
    in_maps = [
        {"x": x[c * NPER:(c + 1) * NPER], **shared} for c in range(NCORES)
    ]
    res = run_bass_kernel_spmd(nc, in_maps, core_ids=list(range(NCORES)))
    out = np.concatenate([r["out"] for r in res.results], axis=0)
    return out.astype(np.float32)


# revision 24
# speedup vs baseline: 1.1908x; 1.0617x over previous
"""Trainium2 Bass kernel for a binarized (XNOR-style) ResNet BasicBlock.

Reference semantics (per nn_BasicBlock_37228776522124):
    out = BN2(conv3x3(sign(BN1(conv3x3(sign(x), sign(w1)*a1))), sign(w2)*a2)) + x
with training-mode BN (batch stats over N,H,W) and per-out-channel
weight scale a_l = mean(|w_l|).

Key facts exploited:
  * conv inputs are exactly +-1 -> fp8 DoubleRow matmuls accumulate EXACT
    integers in fp32 PSUM (|z| <= 2304 < 2^24).
  * conv(sign(x), sign(w)*a) = a * conv(sign(x), sign(w)); a and BN fold
    into one per-channel affine s*z + b applied post-conv.
  * z is always even; z/2 <= 1152 is stored exactly in fp16. conv1's z
    only feeds sign(z - mean), so it is stored as fp8 at z/16.
  * Data-parallel over batch (4 images/core on 8 cores); BN batch stats
    need one AllReduce of [128,4] fp32 per conv.

Pipeline layout (v3):
  * startup: x(n0), w1 chunks ride three parallel DMA paths (sync /
    scalar / gpsimd-SWDGE); weight-sign evacuation runs on the DVE
    (2-op is_ge*2-1) so the ACT queue goes straight to the conv fills.
  * identity residual: x is copied DRAM->DRAM into `out` during conv1;
    the finalize DMA-accumulates s2*z2+b2 on top (gpsimd accum_op=add),
    so no x bytes cross SBUF in the tail.
  * finalize is chunked in 16 half-tiles alternating ACT activation and
    DVE tensor_scalar for the affine, 6-deep buffering.

Self-contained: only needs /opt/trn_rl_repo (the Bass toolchain) + numpy.
"""

import os
import sys

for _p in ("/opt/trn_rl_repo",):
    if os.path.isdir(_p) and _p not in sys.path:
        sys.path.insert(0, _p)

import numpy as np

# Problem shapes (hardcoded per spec)
N_FULL, C, H, W = 32, 256, 56, 56
NCORES = 8
NPER = N_FULL // NCORES          # 4 images per core
SP = H * W                       # 3136
HSP = SP // 2                    # finalize half-tile
HP = H + 2                       # 58 (zero-padded)
SPP = HP * HP                    # 3364
NIB = C // 128                   # 2 input-channel blocks
NOB = C // 128                   # 2 output-channel blocks
NTAP = 9
NK = NTAP * NIB                  # 18 accumulation steps per output tile
RB = 7                           # row-blocks of 8 rows
RBW = 8 * W                      # 448 valid outputs per row-block
NMOV = 8 * HP                    # 464 moving columns (8 contiguous pad rows)
RBQ = NMOV + 2                   # 466 f32 <= one psum bank
EPS = 1e-5
KELEM = C * NTAP                 # 2304 weight elems per out channel
KH = KELEM // 2                  # ib-half of a weight row (contiguous)

_nc_cache = {}


def build_nc(num_devices=NCORES):
    import concourse.bacc as bacc
    import concourse.tile as tile
    import concourse.mybir as mybir
    from concourse.masks import make_identity

    F32 = mybir.dt.float32
    F16 = mybir.dt.float16
    BF16 = mybir.dt.bfloat16
    ALU = mybir.AluOpType
    ACTF = mybir.ActivationFunctionType
    AX = mybir.AxisListType

    nc = bacc.Bacc(
        "TRN2", target_bir_lowering=False, debug=False,
        num_devices=num_devices,
    )

    x_t = nc.dram_tensor("x", [NPER, C, H, W], F32, kind="ExternalInput")
    w_t = [
        nc.dram_tensor("w1", [C, C, 3, 3], F32, kind="ExternalInput"),
        nc.dram_tensor("w2", [C, C, 3, 3], F32, kind="ExternalInput"),
    ]
    g_t = [
        nc.dram_tensor("gamma1", [C], F32, kind="ExternalInput"),
        nc.dram_tensor("gamma2", [C], F32, kind="ExternalInput"),
    ]
    b_t = [
        nc.dram_tensor("beta1", [C], F32, kind="ExternalInput"),
        nc.dram_tensor("beta2", [C], F32, kind="ExternalInput"),
    ]
    out_t = nc.dram_tensor("out", [NPER, C, H, W], F32, kind="ExternalOutput")

    x_ap = x_t.ap().rearrange("n c h w -> n c (h w)")      # [4, 256, 3136]
    out_ap = out_t.ap().rearrange("n c h w -> n c (h w)")
    rgroups = [list(range(num_devices))]
    M_TOTAL = float(num_devices * NPER * SP)
    # debug bisection: W < C1 < AR1 < C2 < FULL
    phase_lim = {"W": 0, "C1": 1, "AR1": 2, "C2": 3, "FULL": 9}[
        os.environ.get("KERNEL_PHASES", "FULL")]
    use_fp8 = os.environ.get("KERNEL_FP8", "1") == "1"
    A8 = mybir.dt.float8e4
    PM = mybir.MatmulPerfMode
    # abuf block pitch: 2-col left margin (first-tap 466-wide matmul reads
    # from grid-2) + 3364 grid + tail pad; 3376 keeps fp8 pair-step 16B-aligned
    ABW = 3376
    GB = 2                          # grid base offset inside each block
    ABD = A8 if use_fp8 else BF16

    with tile.TileContext(nc) as tc:
        with (
            tc.tile_pool(name="consts", bufs=1) as pc,
            tc.tile_pool(name="dbl", bufs=2) as pd,
            tc.tile_pool(name="psum", bufs=8, space="PSUM") as pp,
            tc.tile_pool(name="dram", bufs=1, space="DRAM") as pdram,
        ):
            ident = pc.tile([128, 128], F32, name="ident", tag="ident")
            make_identity(nc, ident[:])
            epsap = pc.tile([128, 1], F32, name="epsap", tag="epsap")
            nc.vector.memset(epsap[:], EPS)

            # persistent stores: z1 as fp8 at z/16 (only feeds sign(z-mean)),
            # z2 as fp16 at z/2 (exact, feeds the output).
            zstore = [
                pc.tile([128, NPER * NOB * SP], A8 if l == 0 else F16,
                        name=f"z{l}", tag=f"z{l}")
                for l in range(2)
            ]
            zscale = [1.0 / 16.0, 0.5]
            wsign = [
                pc.tile([128, NK * NOB * 128], ABD, name=f"ws{l}", tag=f"ws{l}")
                for l in range(2)
            ]
            # full residual x resident as fp16 (exact to ~5e-4 rel; the
            # identity add tolerance is ~2e-3): loaded by gpsimd casting
            # DMAs paced through conv2, so the finalize only WRITES HBM.
            xf16 = pc.tile([128, NPER * NOB * SP], F16, name="xf16",
                           tag="xf16")
            alphar = [pc.tile([128, NOB], F32, name=f"al{l}", tag=f"al{l}") for l in range(2)]
            sumc = [pc.tile([128, NOB * 28], F32, name=f"sc{l}", tag=f"sc{l}") for l in range(2)]
            sqc = [pc.tile([128, NOB * 28], F32, name=f"qc{l}", tag=f"qc{l}") for l in range(2)]
            statloc = [pc.tile([128, 4], F32, name=f"sl{l}", tag=f"sl{l}") for l in range(2)]
            statg = [pc.tile([128, 4], F32, name=f"sg{l}", tag=f"sg{l}") for l in range(2)]
            arin = [[None] * NOB for _ in range(2)]
            arout = [[None] * NOB for _ in range(2)]
            for l in range(2):
                for ob in range(NOB):
                    arin[l][ob] = pdram.tile(
                        [128, 2], F32, name=f"ari{l}{ob}", tag=f"ari{l}{ob}")
                    arout[l][ob] = pdram.tile(
                        [128, 2], F32, name=f"aro{l}{ob}", tag=f"aro{l}{ob}")
            gb = [pc.tile([128, 2 * NOB], F32, name=f"gb{l}", tag=f"gb{l}") for l in range(2)]
            coef = [pc.tile([128, 2 * NOB], F32, name=f"cf{l}", tag=f"cf{l}") for l in range(2)]
            btmp = [pc.tile([128, 14], F32, name=f"bt{l}", tag=f"bt{l}") for l in range(2)]

            # dummy AllReduce at kernel start: absorbs the first-collective
            # latency concurrently with conv1 so the real AR1 is fast
            ard_i = pdram.tile([128, 1], F32, name="ard_i", tag="ard_i")
            ard_o = pdram.tile([128, 1], F32, name="ard_o", tag="ard_o")
            nc.sync.dma_start(ard_i[:], g_t[0].ap()[0:128])
            nc.gpsimd.collective_compute(
                "AllReduce", ALU.add, replica_groups=rgroups,
                ins=[ard_i.opt()], outs=[ard_o.opt()],
            )
            # park the (unused) result in a spare btmp column so DCE keeps it
            nc.gpsimd.dma_start(btmp[0][:, 12:13], ard_o[:])

            # ---------- startup DMAs on three parallel queues ----------
            HROW = H // 2  # 28 rows per half

            def fill1(n, abuf):
                # x in half-tiles through a 3-deep chunk pool: finer DMA
                # pacing and the sign pass starts after the first half
                for hh in range(2):
                    for ib in range(NIB):
                        a58 = abuf[:, ib * ABW + GB:ib * ABW + GB + SPP
                                   ].rearrange("p (h w) -> p h w", w=HP)
                        xc = pd.tile([128, HSP], F32, name="xin", tag="xin",
                                     bufs=3)
                        nc.sync.dma_start(
                            xc[:], x_ap[n, ib * 128:(ib + 1) * 128,
                                        hh * HSP:(hh + 1) * HSP])
                        xv = xc[:].rearrange("p (h w) -> p h w", w=W)
                        nc.scalar.activation(
                            out=a58[:, 1 + hh * HROW:1 + (hh + 1) * HROW,
                                    1:W + 1],
                            in_=xv, func=ACTF.Sign)

            # scalar: w1 ob0 in ib-halves; gpsimd (SWDGE): w1 ob1 halves
            wd0 = w_t[0].ap().rearrange("o i h w -> o (i h w)")
            wraw0 = []
            for ob in range(NOB):
                wr = pc.tile([128, KELEM], F32, name="wraw", tag="wraw",
                             bufs=2)
                eng = nc.scalar if ob == 0 else nc.gpsimd
                for ibh in range(2):
                    eng.dma_start(
                        wr[:, ibh * KH:(ibh + 1) * KH],
                        wd0[ob * 128:(ob + 1) * 128,
                            ibh * KH:(ibh + 1) * KH])
                wraw0.append(wr)
            for ob in range(NOB):
                nc.scalar.dma_start(
                    gb[0][:, ob:ob + 1], g_t[0].ap()[ob * 128:(ob + 1) * 128])
                nc.scalar.dma_start(
                    gb[0][:, NOB + ob:NOB + ob + 1],
                    b_t[0].ap()[ob * 128:(ob + 1) * 128])

            # ---------------- weight prep helpers ----------------
            def wprep_tap(l, wraw, ob, t, ib, dve=False):
                """transpose one (ob, t, ib) 128x128 block + sign-evacuate.

                dve=True: 2-op DVE evacuation (is_ge -> {0,1}, then *2-1),
                keeps the ACT queue free. dve=False: single ACT Sign."""
                wtap = wraw[:].rearrange("p (i t) -> p t i", t=NTAP)
                if use_fp8:
                    kidx = (ob * NTAP + t) * 2 + ib
                else:
                    kidx = ob * NK + t * NIB + ib
                dst = wsign[l][:, kidx * 128:(kidx + 1) * 128]
                psT = pp.tile([128, RBW], F32, name="cps", tag="cps")
                nc.tensor.transpose(
                    psT[:, 0:128],
                    wtap[:, t, ib * 128:(ib + 1) * 128],
                    ident[:],
                )
                if dve:
                    # {0,2} = (wT >= 0)*2, then in-place -1 -> exact +-1 fp8
                    nc.vector.tensor_scalar(
                        out=dst, in0=psT[:, 0:128],
                        scalar1=0.0, scalar2=2.0, op0=ALU.is_ge, op1=ALU.mult,
                    )
                    nc.vector.tensor_scalar_add(dst, dst, -1.0)
                else:
                    nc.scalar.activation(
                        out=dst, in_=psT[:, 0:128], func=ACTF.Sign)

            def wprep_alpha(l, wraw, ob):
                nc.vector.tensor_reduce(
                    out=alphar[l][:, ob:ob + 1], in_=wraw[:],
                    axis=AX.X, op=ALU.add, apply_absolute_value=True,
                )

            # conv1 weight prep: transposes paced by DVE sign-evacuation,
            # ordered by DMA chunk arrival (ob0ib0, ob0ib1, ob1ib0, ob1ib1)
            for ob in range(NOB):
                for ib in range(NIB):
                    for t in range(NTAP):
                        wprep_tap(0, wraw0[ob], ob, t, ib, dve=True)

            # fill1 for n=0 (emitted now; ACT goes straight to it while
            # the DVE paces the weight-sign evacuation)
            abuf0 = pd.tile([128, NIB * ABW], ABD, name="abuf", tag="abuf")
            for ib in range(NIB):
                a58 = abuf0[:, ib * ABW + GB:ib * ABW + GB + SPP
                            ].rearrange("p (h w) -> p h w", w=HP)
                nc.vector.memset(a58[:, 0:1, :], 0.0)
                nc.vector.memset(a58[:, HP - 1:HP, :], 0.0)
                nc.vector.memset(a58[:, :, 0:1], 0.0)
                nc.vector.memset(a58[:, :, HP - 1:HP], 0.0)
                nc.vector.memset(abuf0[:, ib * ABW:ib * ABW + GB], 0.0)
                nc.vector.memset(
                    abuf0[:, ib * ABW + GB + SPP:(ib + 1) * ABW], 0.0)
            fill1(0, abuf0)

            for ob in range(NOB):
                wprep_alpha(0, wraw0[ob], ob)

            # ---------------- one conv pass (shared for conv1/conv2) --------
            def conv_pass(l, act_fill, do_ar=True, abuf_pre=None,
                          post_group=None, ob_major=False):
                """act_fill(n, abuf) writes signed acts into the padded
                [128, NIB*SPP] buffer interior (ring already zero).
                post_group(n, ob) emits extra work after each group."""
                def make_abuf(n):
                    abuf = pd.tile([128, NIB * ABW], ABD, name="abuf",
                                   tag="abuf")
                    for ib in range(NIB):
                        a58 = abuf[:, ib * ABW + GB:ib * ABW + GB + SPP
                                   ].rearrange("p (h w) -> p h w", w=HP)
                        nc.vector.memset(a58[:, 0:1, :], 0.0)
                        nc.vector.memset(a58[:, HP - 1:HP, :], 0.0)
                        nc.vector.memset(a58[:, :, 0:1], 0.0)
                        nc.vector.memset(a58[:, :, HP - 1:HP], 0.0)
                        nc.vector.memset(abuf[:, ib * ABW:ib * ABW + GB], 0.0)
                        nc.vector.memset(
                            abuf[:, ib * ABW + GB + SPP:(ib + 1) * ABW], 0.0)
                    act_fill(n, abuf)
                    return abuf

                def emit_group(n, ob, abuf):
                        ps = [pp.tile([128, RBQ], F32, name="cps", tag="cps")
                              for _ in range(RB)]
                        if use_fp8:
                            ab3 = abuf[:].rearrange(
                                "p (two s) -> p two s", two=NIB)
                            for t in range(NTAP):
                                th, tw = t // 3, t % 3
                                base = (ob * NTAP + t) * 2 * 128
                                lhsT = wsign[l][:, base:base + 256].rearrange(
                                    "p (two m) -> p two m", two=2)
                                for rb in range(RB):
                                    r0 = (rb * 8 + th) * HP
                                    if t == 0:
                                        rhs = ab3[:, :, r0:r0 + RBQ]
                                        outap = ps[rb][:, 0:RBQ]
                                    else:
                                        rhs = ab3[:, :, GB + r0:GB + r0 + NMOV]
                                        outap = ps[rb][:, 2 - tw:2 - tw + NMOV]
                                    nc.tensor.matmul(
                                        outap, lhsT, rhs,
                                        start=(t == 0), stop=(t == NTAP - 1),
                                        perf_mode=PM.DoubleRow,
                                    )
                        else:
                            for k in range(NK):
                                t, ib = k // NIB, k % NIB
                                th, tw = t // 3, t % 3
                                kidx = ob * NK + t * NIB + ib
                                af = abuf[:, ib * ABW:(ib + 1) * ABW]
                                lhsT = wsign[l][:, kidx * 128:(kidx + 1) * 128]
                                for rb in range(RB):
                                    r0 = (rb * 8 + th) * HP
                                    if k == 0:
                                        rhs = af[:, r0:r0 + RBQ]
                                        outap = ps[rb][:, 0:RBQ]
                                    else:
                                        rhs = af[:, GB + r0:GB + r0 + NMOV]
                                        outap = ps[rb][:, 2 - tw:2 - tw + NMOV]
                                    nc.tensor.matmul(
                                        outap, lhsT, rhs,
                                        start=(k == 0),
                                        stop=(k == NK - 1),
                                    )
                        zs = zstore[l]
                        for rb in range(RB):
                            col = n * RB + rb
                            zsl = zs[:, ((n * NOB + ob) * SP + rb * RBW):
                                      ((n * NOB + ob) * SP + (rb + 1) * RBW)
                                      ].rearrange("p (h w) -> p h w", w=W)
                            qv = ps[rb][:, 2:2 + NMOV].rearrange(
                                "p (h w) -> p h w", w=HP)[:, :, 0:W]
                            # z*zscale -> store on DVE; accum_out = sum
                            nc.vector.tensor_scalar(
                                out=zsl, in0=qv,
                                scalar1=zscale[l], scalar2=None, op0=ALU.mult,
                                op1=ALU.add,
                                accum_out=sumc[l][:, ob * 28 + col:
                                                  ob * 28 + col + 1],
                            )
                            # scr = z^2 (bf16 dummy out); accum = sum(z^2)
                            scr = pd.tile([128, RBW], BF16, name="scr",
                                          tag="scr")
                            nc.scalar.activation(
                                out=scr[:].rearrange("p (h w) -> p h w", w=W),
                                in_=qv, func=ACTF.Square,
                                accum_out=sqc[l][:, ob * 28 + col:
                                                 ob * 28 + col + 1],
                            )
                        if post_group is not None:
                            post_group(n, ob)
                        if do_ar and n == NPER - 1:
                            stats_ar(l, ob)

                if ob_major:
                    # all ob0 groups first: AR-a + its BN fold complete
                    # DURING the ob1 half, so the first 8 finalize chunks
                    # are ungated when the conv ends. Each group gets its
                    # own freshly-filled abuf (fills read SBUF, cheap).
                    glist = [(ob, n) for ob in range(NOB)
                             for n in range(NPER)]
                    pend_ab = None
                    for gi, (ob, n) in enumerate(glist):
                        abuf = pend_ab if pend_ab is not None \
                            else make_abuf(n)
                        pend_ab = (make_abuf(glist[gi + 1][1])
                                   if gi + 1 < len(glist) else None)
                        emit_group(n, ob, abuf)
                else:
                    pend = {}
                    for n in range(NPER):
                        if n == 0 and abuf_pre is not None:
                            abuf = abuf_pre
                        elif n in pend:
                            abuf = pend.pop(n)
                        else:
                            abuf = make_abuf(n)
                        # emit the NEXT image's fill ahead of this image's
                        # drains so the ACT queue never parks it behind
                        # the PE-gated square passes
                        if n + 1 < NPER:
                            pend[n + 1] = make_abuf(n + 1)
                        for ob in range(NOB):
                            emit_group(n, ob, abuf)
                if do_ar:
                    return
                # debug path (no AR): just the local reduces
                for ob in range(NOB):
                    nc.vector.tensor_reduce(
                        out=statloc[l][:, 2 * ob:2 * ob + 1],
                        in_=sumc[l][:, ob * 28:(ob + 1) * 28],
                        axis=AX.X, op=ALU.add,
                    )
                    nc.vector.tensor_reduce(
                        out=statloc[l][:, 2 * ob + 1:2 * ob + 2],
                        in_=sqc[l][:, ob * 28:(ob + 1) * 28],
                        axis=AX.X, op=ALU.add,
                    )

            def stats_ar(l, ob):
                """per-ob stats reduce + AllReduce + BN fold: ob0's
                collective runs while the PE still computes ob1's last
                group, and downstream per-ob consumers unblock early."""
                nc.vector.tensor_reduce(
                    out=statloc[l][:, 2 * ob:2 * ob + 1],
                    in_=sumc[l][:, ob * 28:(ob + 1) * 28],
                    axis=AX.X, op=ALU.add,
                )
                nc.vector.tensor_reduce(
                    out=statloc[l][:, 2 * ob + 1:2 * ob + 2],
                    in_=sqc[l][:, ob * 28:(ob + 1) * 28],
                    axis=AX.X, op=ALU.add,
                )
                # AR DMAs: sync for l=0, scalar for l=1
                dma_eng = nc.sync if l == 0 else nc.scalar
                dma_eng.dma_start(arin[l][ob][:],
                                  statloc[l][:, 2 * ob:2 * ob + 2])
                nc.gpsimd.collective_compute(
                    "AllReduce", ALU.add, replica_groups=rgroups,
                    ins=[arin[l][ob].opt()], outs=[arout[l][ob].opt()],
                )
                dma_eng.dma_start(statg[l][:, 2 * ob:2 * ob + 2],
                                  arout[l][ob][:])
                # BN fold: coef = [s' | beta - s*mean] for this ob
                tmp = btmp[l]
                mean = tmp[:, 0 + ob * 6:1 + ob * 6]
                e2 = tmp[:, 1 + ob * 6:2 + ob * 6]
                var = tmp[:, 2 + ob * 6:3 + ob * 6]
                alp = tmp[:, 3 + ob * 6:4 + ob * 6]
                tt = tmp[:, 4 + ob * 6:5 + ob * 6]
                std = tmp[:, 5 + ob * 6:6 + ob * 6]
                nc.vector.tensor_scalar_mul(
                    mean, statg[l][:, 2 * ob:2 * ob + 1],
                    1.0 / (zscale[l] * M_TOTAL))
                nc.vector.tensor_scalar_mul(
                    e2, statg[l][:, 2 * ob + 1:2 * ob + 2], 1.0 / M_TOTAL)
                nc.vector.tensor_mul(var, mean, mean)
                nc.vector.tensor_sub(var, e2, var)
                nc.vector.tensor_scalar_mul(
                    alp, alphar[l][:, ob:ob + 1], 1.0 / KELEM)
                nc.vector.tensor_mul(tt, alp, alp)
                nc.vector.tensor_mul(tt, tt, var)
                nc.scalar.activation(std, tt, ACTF.Sqrt, bias=epsap[:])
                nc.vector.reciprocal(tt, std)
                nc.vector.tensor_mul(tt, tt, alp)             # alpha*inv
                nc.vector.tensor_mul(tt, tt, gb[l][:, ob:ob + 1])  # *gamma
                nc.vector.tensor_scalar_mul(
                    coef[l][:, ob:ob + 1], tt, 1.0 / zscale[l])
                nc.vector.tensor_mul(tt, tt, mean)
                nc.vector.tensor_sub(
                    coef[l][:, NOB + ob:NOB + ob + 1],
                    gb[l][:, NOB + ob:NOB + ob + 1], tt)

            # ---------------- conv1: acts = sign(x) ----------------
            if phase_lim >= 1:
                conv_pass(0, fill1, do_ar=(phase_lim >= 2), abuf_pre=abuf0)

            # conv2 weight prep: PE transposes + ACT signs queue up behind
            # conv1's work and execute inside the AR1 bubble.
            wd1 = w_t[1].ap().rearrange("o i h w -> o (i h w)")
            wraw1 = []
            for ob in range(NOB):
                wr = pc.tile([128, KELEM], F32, name="wraw", tag="wraw",
                             bufs=2)
                nc.sync.dma_start(wr[:], wd1[ob * 128:(ob + 1) * 128, :])
                wraw1.append(wr)
            for ob in range(NOB):
                nc.sync.dma_start(
                    gb[1][:, ob:ob + 1], g_t[1].ap()[ob * 128:(ob + 1) * 128])
                nc.sync.dma_start(
                    gb[1][:, NOB + ob:NOB + ob + 1],
                    b_t[1].ap()[ob * 128:(ob + 1) * 128])
            for ob in range(NOB):
                for ib in range(NIB):
                    for t in range(NTAP):
                        wprep_tap(1, wraw1[ob], ob, t, ib, dve=False)
                wprep_alpha(1, wraw1[ob], ob)

            # ---------------- conv2: acts = sign(s1*z1 + b1) ----------------
            def fill2(n, abuf):
                # n=0 emitted in row-halves: the first matmul group only
                # needs the top rows of both ib blocks
                halves = 2 if n == 0 else 1
                HR = H // halves
                for hh in range(halves):
                    for ib in range(NIB):
                        a58 = abuf[:, ib * ABW + GB:ib * ABW + GB + SPP
                                   ].rearrange("p (h w) -> p h w", w=HP)
                        zv = zstore[0][:, (n * NOB + ib) * SP:
                                       (n * NOB + ib + 1) * SP].rearrange(
                            "p (h w) -> p h w", w=W)
                        nc.scalar.activation(
                            out=a58[:, 1 + hh * HR:1 + (hh + 1) * HR,
                                    1:W + 1],
                            in_=zv[:, hh * HR:(hh + 1) * HR, :],
                            func=ACTF.Sign,
                            scale=coef[0][:, ib:ib + 1],
                            bias=coef[0][:, NOB + ib:NOB + ib + 1],
                        )

            # residual prefetch: cast-DMA x (f32 DRAM -> fp16 SBUF) on
            # the gpsimd SWDGE queue, paced one tile per conv2 group so it
            # never congests the AR windows or the HWDGE rings.
            def xf16_load(n, ob):
                if phase_lim >= 9:
                    seg = (n * NOB + ob) * SP
                    nc.gpsimd.dma_start(
                        xf16[:, seg:seg + SP],
                        x_ap[n, ob * 128:(ob + 1) * 128, :])

            if phase_lim >= 3:
                conv_pass(1, fill2, do_ar=(phase_lim >= 9),
                          post_group=xf16_load, ob_major=True)

            if phase_lim < 9:
                # debug: dump something touching live tiles into out
                dbg = pd.tile([128, SP], F32, name="dbg", tag="dbg")
                if phase_lim >= 1:
                    nc.vector.tensor_copy(dbg[:], zstore[0][:, 0:SP])
                else:
                    nc.vector.tensor_copy(dbg[:], wsign[0][:, 0:SP])
                nc.sync.dma_start(out_ap[0, 0:128, :], dbg[:])

            # ------------- finalize: out = s2*z2 + b2 + x --------------
            # 16 half-tiles: ACT affine -> DVE add (x from resident fp16)
            # -> HWDGE write. Tail HBM traffic is the 12.8MB of writes only.
            NCH = 2 * NPER * NOB
            for k in range(NCH if phase_lim >= 9 else 0):
                # ob-major: the ob0 chunks only need AR2a, so their writes
                # stream while ob1's collective is still completing
                ob, n, hh = k // 8, (k % 8) // 2, k % 2
                zoff = (n * NOB + ob) * SP + hh * HSP
                t1 = pd.tile([128, HSP], F32, name="t1ch", tag="t1ch",
                             bufs=3)
                nc.scalar.activation(
                    out=t1[:],
                    in_=zstore[1][:, zoff:zoff + HSP],
                    func=ACTF.Identity,
                    scale=coef[1][:, ob:ob + 1],
                    bias=coef[1][:, NOB + ob:NOB + ob + 1],
                )
                nc.vector.tensor_add(t1[:], t1[:], xf16[:, zoff:zoff + HSP])
                nc.sync.dma_start(
                    out_ap[n, ob * 128:(ob + 1) * 128,
                           hh * HSP:(hh + 1) * HSP], t1[:])

    nc.compile()
    return nc


def _get_nc(num_devices=NCORES):
    if num_devices not in _nc_cache:
        _nc_cache[num_devices] = build_nc(num_devices)
    return _nc_cache[num_devices]


def kernel(**inputs):
    from concourse.bass_utils import run_bass_kernel_spmd

    nc = _get_nc(NCORES)
    x = np.ascontiguousarray(np.asarray(inputs["x"], dtype=np.float32))
    shared = {
        k: np.ascontiguousarray(np.asarray(inputs[k], dtype=np.float32))
        for k in ("w1", "gamma1", "beta1", "w2", "gamma2", "beta2")
    }
    in_maps = [
        {"x": x[c * NPER:(c + 1) * NPER], **shared} for c in range(NCORES)
    ]
    res = run_bass_kernel_spmd(nc, in_maps, core_ids=list(range(NCORES)))
    out = np.concatenate([r["out"] for r in res.results], axis=0)
    return out.astype(np.float32)
